# revision 14
# baseline (speedup 1.0000x reference)
"""Trainium2 Bass kernel for nn_AttentionBlock (B=16, C=512, H=W=32, 8 heads, d_k=64).

Sharding: data-parallel over batch; each of the 8 NeuronCores computes 2 batches.

Per batch (matmuls f32r, K=128, base partition 0, free <= 512):
  qkT projection  : qpair[p]  = [q_{2p}; q_{2p+1}]^T  (128 x 1024, channels on partitions)
                    kpadA/B[p] = k of one head zero-padded to 128 rows (scores lhsT)
                    per-partition qk bias folded into the PSUM->SBUF copy (tensor_scalar)
  v projection    : v_aug[:, t, h, 0:64] = v tokens, [..., 64] = 1.0; free-dim bias
                    added via a broadcast bias tile in the PSUM->SBUF copy
  attention       : scoresT = kpad.T @ qpair ; expT = exp(scale*scoresT) (ACT -> f32r)
                    [res; sumexp] = [v|1].T @ expT   (M=65 matmul, accum over j tiles)
                    res_norm = res / sumexp          (DVE divide, sums broadcast via DRAM)
  out projection  : outT = W_out.T-chunks @ res_norm + (x + b_out)  (residual+bias host-fused)
"""
import numpy as np

import concourse.bass as bass
from concourse import bacc
import concourse.mybir as mybir
import concourse.tile as tile
from concourse import bass_utils

F32 = mybir.dt.float32
F32R = mybir.dt.float32r
AF = mybir.ActivationFunctionType
ALU = mybir.AluOpType

N_HEADS = 8
DK = 64
SCALE = DK ** -0.5
C = 512
N = 1024            # tokens per batch (32*32)
NB = 2              # batches per core
NCORES = 8
NCH = C // 128      # 4 contraction chunks
NT = N // 128       # 8 token tiles
NPAIR = N_HEADS // 2


def build():
    nc = bacc.Bacc(None, target_bir_lowering=False, num_swdge_queues=4)
    x_d = nc.dram_tensor("x", (NB, C, N), F32, kind="ExternalInput")
    xpb_d = nc.dram_tensor("xpb", (NB, C, N), F32, kind="ExternalInput")
    wqk_d = nc.dram_tensor("w_qk", (C, 2, NPAIR, 128), F32, kind="ExternalInput")
    bqkt_d = nc.dram_tensor("b_qk_t", (128, 2, NPAIR), F32, kind="ExternalInput")
    wv_d = nc.dram_tensor("w_v", (C, C), F32, kind="ExternalInput")
    bv_d = nc.dram_tensor("b_v", (1, C), F32, kind="ExternalInput")
    wout_d = nc.dram_tensor("w_out", (C, C), F32, kind="ExternalInput")
    y_d = nc.dram_tensor("y", (NB, C, N), F32, kind="ExternalOutput")

    with tile.TileContext(nc) as tc:
        with (
            tc.tile_pool(name="const", bufs=1) as const,
            tc.tile_pool(name="persist", bufs=1) as persist,
            tc.tile_pool(name="sbwork", bufs=3) as sbwork,
            tc.tile_pool(name="sbexp", bufs=4) as sbexp,
            tc.tile_pool(name="ps_big", bufs=2, space="PSUM") as ps_big,
            tc.tile_pool(name="ps_res", bufs=4, space="PSUM") as ps_res,
            tc.tile_pool(name="dram", bufs=8, space="DRAM") as dram,
        ):
            # ---- constants / weights (one-time) ----
            # order matters: first batch's x and the qk weights load first so
            # the PE can start the first projection ASAP.
            x_r = [persist.tile([128, N], F32R, name=f"xr{ch}")
                   for ch in range(NCH)]
            wqk = []
            for ch in range(NCH):
                w = const.tile([128, 2, NPAIR, 128], F32R, name=f"wqk{ch}")
                nc.gpsimd.dma_start(w[:, 0], wqk_d[ch * 128:(ch + 1) * 128, 0])
                nc.gpsimd.dma_start(x_r[ch][:], x_d[0, ch * 128:(ch + 1) * 128, :])
                nc.gpsimd.dma_start(w[:, 1], wqk_d[ch * 128:(ch + 1) * 128, 1])
                wqk.append(w)
            wv = []
            for ch in range(NCH):
                w2 = const.tile([128, C], F32R, name=f"wv{ch}")
                nc.gpsimd.dma_start(w2[:], wv_d[ch * 128:(ch + 1) * 128, :])
                wv.append(w2)
            wo = []
            for ch in range(NCH):
                w = const.tile([128, C], F32R, name=f"wout{ch}")
                nc.gpsimd.dma_start(w[:], wout_d[ch * 128:(ch + 1) * 128, :])
                wo.append(w)
            bqkt = const.tile([128, 2, NPAIR], F32)
            nc.sync.dma_start(bqkt[:], bqkt_d[:])
            bv_bc = const.tile([128, C], F32)   # b_v broadcast to all partitions
            nc.sync.dma_start(bv_bc[:], bv_d[:].to_broadcast([128, C]))

            ones_f = const.tile([128, 8], F32)
            nc.vector.memset(ones_f[:], 1.0)
            zeros_f = const.tile([64, N], F32)
            nc.vector.memset(zeros_f[:], 0.0)

            # ---- persistent per-batch buffers ----
            qpair = [persist.tile([128, N], F32R, name=f"qpair{p}")
                     for p in range(NPAIR)]
            kpad = [[persist.tile([128, N], F32R, name=f"kpad{p}_{s}")
                     for s in range(2)] for p in range(NPAIR)]
            # zero the pad halves once; they are never overwritten
            for p in range(NPAIR):
                nc.vector.tensor_copy(kpad[p][0][64:128, :], zeros_f[:])
                nc.vector.tensor_copy(kpad[p][1][0:64, :], zeros_f[:])
            v_aug = persist.tile([128, NT, N_HEADS, DK + 1], F32R)
            res_all = [persist.tile([128, N], F32R, name=f"resall{p}")
                       for p in range(NPAIR)]

            for b in range(NB):
                if b > 0:
                    with nc.named_scope(f"b{b}_load"):
                        for ch in range(NCH):
                            nc.gpsimd.dma_start(
                                x_r[ch][:], x_d[b, ch * 128:(ch + 1) * 128, :])

                with nc.named_scope(f"b{b}_qkt"):
                    for p in range(NPAIR):
                        for qk in range(2):   # 0 = q, 1 = k
                            ps = ps_big.tile([128, N], F32, tag="big", name="qk_ps")
                            for nh in range(2):
                                nsl = slice(nh * 512, nh * 512 + 512)
                                for ch in range(NCH):
                                    nc.tensor.matmul(
                                        ps[:, nsl], wqk[ch][:, qk, p, :],
                                        x_r[ch][:, nsl],
                                        start=(ch == 0), stop=(ch == NCH - 1))
                            if qk == 0:
                                nc.vector.tensor_scalar(
                                    out=qpair[p][:], in0=ps[:],
                                    scalar1=bqkt[:, 0, p:p + 1],
                                    scalar2=None, op0=ALU.add)
                            else:
                                nc.vector.tensor_scalar(
                                    out=kpad[p][0][0:64, :], in0=ps[0:64, :],
                                    scalar1=bqkt[0:64, 1, p:p + 1],
                                    scalar2=None, op0=ALU.add)
                                nc.vector.tensor_scalar(
                                    out=kpad[p][1][64:128, :], in0=ps[64:128, :],
                                    scalar1=bqkt[64:128, 1, p:p + 1],
                                    scalar2=None, op0=ALU.add)

                with nc.named_scope(f"b{b}_v"):
                    for t in range(NT):
                        ps = ps_big.tile([128, N], F32, tag="big", name="v_ps")
                        for ch in range(NCH):
                            nc.tensor.matmul(
                                ps[:, 0:512], x_r[ch][:, t * 128:(t + 1) * 128],
                                wv[ch][:], start=(ch == 0), stop=(ch == NCH - 1))
                        nc.vector.tensor_add(
                            v_aug[:, t, :, 0:DK],
                            ps[:, 0:512].rearrange("p (h d) -> p h d", h=N_HEADS),
                            bv_bc[:].rearrange("p (h d) -> p h d", h=N_HEADS))
                        nc.vector.tensor_copy(
                            v_aug[:, t, :, DK:DK + 1], ones_f[:].unsqueeze(2))

                for p in range(NPAIR):
                    with nc.named_scope(f"b{b}_attn{p}"):
                        for ic in range(2):
                            isl = slice(ic * 512, ic * 512 + 512)
                            res_ps = [ps_res.tile([DK + 1, 512], F32, tag="res",
                                                  name=f"res_ps{s}")
                                      for s in range(2)]
                            for t in range(NT):
                                js = slice(t * 128, (t + 1) * 128)
                                s_ps = ps_big.tile([128, N], F32, tag="big",
                                                   name="s_ps")
                                nc.tensor.matmul(
                                    s_ps[:, 0:512], kpad[p][0][:, js],
                                    qpair[p][:, isl], start=True, stop=True)
                                nc.tensor.matmul(
                                    s_ps[:, 512:1024], kpad[p][1][:, js],
                                    qpair[p][:, isl], start=True, stop=True)
                                exp_sb = sbexp.tile([128, 2, 512], F32R, tag="exp",
                                                    name="exp_sb")
                                for s in range(2):
                                    nc.scalar.activation(
                                        out=exp_sb[:, s, :],
                                        in_=s_ps[:, s * 512:(s + 1) * 512],
                                        func=AF.Exp, scale=SCALE)
                                    nc.tensor.matmul(
                                        res_ps[s][:],
                                        v_aug[:, t, 2 * p + s, :],
                                        exp_sb[:, s, :],
                                        start=(t == 0), stop=(t == NT - 1))
                            # normalize: res / sumexp (broadcast row 64 via DRAM)
                            for s in range(2):
                                rcp_sb = sbwork.tile([1, 512], F32, tag="sums",
                                                     name="rcp_sb")
                                sum_sb = sbwork.tile([1, 512], F32, tag="sumsb",
                                                     name="sum_sb")
                                nc.vector.tensor_copy(sum_sb[:],
                                                      res_ps[s][DK:DK + 1, :])
                                nc.vector.reciprocal_approx_fast(
                                    out=rcp_sb[:], in_=sum_sb[:])
                                rcp_dram = dram.tile([1, 512], F32, tag="sumd",
                                                     name="rcp_dram")
                                nc.gpsimd.dma_start(rcp_dram[:], rcp_sb[:])
                                mult = sbwork.tile([64, 512], F32, tag="mult",
                                                   name="mult")
                                nc.gpsimd.dma_start(
                                    mult[:], rcp_dram[:].to_broadcast([64, 512]))
                                nc.vector.tensor_mul(
                                    res_all[p][s * 64:(s + 1) * 64, isl],
                                    res_ps[s][0:DK, :], mult[:])

                with nc.named_scope(f"b{b}_out"):
                    x_res_t = []
                    for ct in range(NCH):
                        xr = sbwork.tile([128, N], F32, tag="xres", bufs=4,
                                         name=f"x_res{ct}")
                        nc.sync.dma_start(xr[:], xpb_d[b, ct * 128:(ct + 1) * 128, :])
                        x_res_t.append(xr)
                    for ctp in range(NCH // 2):
                        cts = [2 * ctp, 2 * ctp + 1]
                        pss = [ps_big.tile([128, N], F32, tag="big",
                                           name=f"out_ps{ct}") for ct in cts]
                        for ch in range(NCH):
                            for i, ct in enumerate(cts):
                                csl = slice(ct * 128, (ct + 1) * 128)
                                for nh in range(2):
                                    nsl = slice(nh * 512, nh * 512 + 512)
                                    nc.tensor.matmul(
                                        pss[i][:, nsl], wo[ch][:, csl],
                                        res_all[ch][:, nsl],
                                        start=(ch == 0), stop=(ch == NCH - 1))
                        for i, ct in enumerate(cts):
                            csl = slice(ct * 128, (ct + 1) * 128)
                            out_sb = sbwork.tile([128, N], F32, tag="out",
                                                 name=f"out_sb{ct}")
                            nc.vector.tensor_add(out_sb[:], pss[i][:], x_res_t[ct][:])
                            nc.sync.dma_start(y_d[b, csl, :], out_sb[:])

    nc.finalize()
    return nc


_NC = None


def _get_nc():
    global _NC
    if _NC is None:
        _NC = build()
    return _NC


def make_in_maps(x, W_qkv, b_qkv, W_out, b_out):
    x = np.ascontiguousarray(np.asarray(x, np.float32)).reshape(16, C, N)
    b_out = np.asarray(b_out, np.float32)
    xpb = np.ascontiguousarray(x + b_out[None, :, None])
    w3 = np.asarray(W_qkv, np.float32).reshape(C, N_HEADS, 3, DK)
    w_qk = np.ascontiguousarray(
        np.stack([w3[:, :, 0], w3[:, :, 1]], axis=1).reshape(C, 2, NPAIR, 128))
    w_v = np.ascontiguousarray(w3[:, :, 2].reshape(C, C))
    b3 = np.asarray(b_qkv, np.float32).reshape(N_HEADS, 3, DK)
    # b_qk_t[j, qk, p] = bias for partition j of pair-tile (qk, p)
    b_qk_t = np.ascontiguousarray(
        np.stack([b3[:, 0], b3[:, 1]], axis=0)      # (2, 8, 64)
        .reshape(2, NPAIR, 128).transpose(2, 0, 1))  # (128, 2, 4)
    b_v = np.ascontiguousarray(b3[:, 2].reshape(1, C))
    maps = []
    for core in range(NCORES):
        maps.append({
            "x": x[core * NB:(core + 1) * NB],
            "xpb": xpb[core * NB:(core + 1) * NB],
            "w_qk": w_qk,
            "b_qk_t": b_qk_t,
            "w_v": w_v,
            "b_v": b_v,
            "w_out": np.asarray(W_out, np.float32),
        })
    return maps


def run_on_hw(in_maps, **kwargs):
    nc = _get_nc()
    return bass_utils.run_bass_kernel_spmd(
        nc, in_maps, core_ids=list(range(NCORES)), **kwargs)


def kernel(x, W_qkv, b_qkv, W_out, b_out):
    res = run_on_hw(make_in_maps(x, W_qkv, b_qkv, W_out, b_out))
    y = np.concatenate([r["y"] for r in res.results], axis=0)  # (16, C, N)
    return y.reshape(16, C, 32, 32).astype(np.float32)


# revision 15
# speedup vs baseline: 1.0801x; 1.0801x over previous
"""Trainium2 Bass kernel for nn_AttentionBlock (B=16, C=512, H=W=32, 8 heads, d_k=64).

Sharding: data-parallel over batch; each of the 8 NeuronCores computes 2 batches.

Per batch (matmuls f32r, K=128, base partition 0, free <= 512):
  qkT projection  : qpair[p]  = [q_{2p}; q_{2p+1}]^T  (128 x 1024, channels on partitions)
                    kpadA/B[p] = k of one head zero-padded to 128 rows (scores lhsT)
                    per-partition qk bias folded into the PSUM->SBUF copy (tensor_scalar)
  v projection    : v_aug[:, t, h, 0:64] = v tokens, [..., 64] = 1.0; free-dim bias
                    added via a broadcast bias tile in the PSUM->SBUF copy
  attention       : scoresT = kpad.T @ qpair ; expT = exp(scale*scoresT) (ACT -> f32r)
                    [res; sumexp] = [v|1].T @ expT   (M=65 matmul, accum over j tiles)
                    res_norm = res / sumexp          (DVE divide, sums broadcast via DRAM)
  out projection  : outT = W_out.T-chunks @ res_norm + (x + b_out)  (residual+bias host-fused)
"""
import numpy as np

import concourse.bass as bass
from concourse import bacc
import concourse.mybir as mybir
import concourse.tile as tile
from concourse import bass_utils

F32 = mybir.dt.float32
F32R = mybir.dt.float32r
AF = mybir.ActivationFunctionType
ALU = mybir.AluOpType

N_HEADS = 8
DK = 64
SCALE = DK ** -0.5
C = 512
N = 1024            # tokens per batch (32*32)
NB = 2              # batches per core
NCORES = 8
NCH = C // 128      # 4 contraction chunks
NT = N // 128       # 8 token tiles
NPAIR = N_HEADS // 2


def build():
    nc = bacc.Bacc(None, target_bir_lowering=False, num_swdge_queues=4)
    x_d = nc.dram_tensor("x", (NB, C, N), F32, kind="ExternalInput")
    xpb_d = nc.dram_tensor("xpb", (NB, C, N), F32, kind="ExternalInput")
    wqk_d = nc.dram_tensor("w_qk", (C, 2, NPAIR, 128), F32, kind="ExternalInput")
    bqkt_d = nc.dram_tensor("b_qk_t", (128, 2, NPAIR), F32, kind="ExternalInput")
    wv_d = nc.dram_tensor("w_v", (C, C), F32, kind="ExternalInput")
    bv_d = nc.dram_tensor("b_v", (1, C), F32, kind="ExternalInput")
    wout_d = nc.dram_tensor("w_out", (C, C), F32, kind="ExternalInput")
    y_d = nc.dram_tensor("y", (NB, C, N), F32, kind="ExternalOutput")

    with tile.TileContext(nc) as tc:
        with (
            tc.tile_pool(name="const", bufs=1) as const,
            tc.tile_pool(name="persist", bufs=1) as persist,
            tc.tile_pool(name="sbwork", bufs=3) as sbwork,
            tc.tile_pool(name="sbexp", bufs=4) as sbexp,
            tc.tile_pool(name="ps_big", bufs=2, space="PSUM") as ps_big,
            tc.tile_pool(name="ps_res", bufs=4, space="PSUM") as ps_res,
            tc.tile_pool(name="dram", bufs=8, space="DRAM") as dram,
        ):
            # ---- constants / weights (one-time) ----
            # order matters: first batch's x and the qk weights load first so
            # the PE can start the first projection ASAP.
            x_r = [persist.tile([128, N], F32R, name=f"xr{ch}")
                   for ch in range(NCH)]
            wqk = []
            for ch in range(NCH):
                w = const.tile([128, 2, NPAIR, 128], F32R, name=f"wqk{ch}")
                nc.gpsimd.dma_start(w[:, 0], wqk_d[ch * 128:(ch + 1) * 128, 0])
                nc.gpsimd.dma_start(x_r[ch][:], x_d[0, ch * 128:(ch + 1) * 128, :])
                nc.gpsimd.dma_start(w[:, 1], wqk_d[ch * 128:(ch + 1) * 128, 1])
                wqk.append(w)
            wv = []
            for ch in range(NCH):
                w2 = const.tile([128, C], F32R, name=f"wv{ch}")
                nc.gpsimd.dma_start(w2[:], wv_d[ch * 128:(ch + 1) * 128, :])
                wv.append(w2)
            wo = []
            for ch in range(NCH):
                w = const.tile([128, C], F32R, name=f"wout{ch}")
                nc.gpsimd.dma_start(w[:], wout_d[ch * 128:(ch + 1) * 128, :])
                wo.append(w)
            bqkt = const.tile([128, 2, NPAIR], F32)
            nc.sync.dma_start(bqkt[:], bqkt_d[:])
            bv_bc = const.tile([128, C], F32)   # b_v broadcast to all partitions
            nc.sync.dma_start(bv_bc[:], bv_d[:].to_broadcast([128, C]))

            ones_f = const.tile([128, 8], F32)
            nc.vector.memset(ones_f[:], 1.0)
            zeros_f = const.tile([64, N], F32)
            nc.vector.memset(zeros_f[:], 0.0)

            # ---- persistent per-batch buffers ----
            qpair = [persist.tile([128, N], F32R, name=f"qpair{p}")
                     for p in range(NPAIR)]
            kpad = [[persist.tile([128, N], F32R, name=f"kpad{p}_{s}")
                     for s in range(2)] for p in range(NPAIR)]
            # zero the pad halves once; they are never overwritten
            for p in range(NPAIR):
                nc.vector.tensor_copy(kpad[p][0][64:128, :], zeros_f[:])
                nc.vector.tensor_copy(kpad[p][1][0:64, :], zeros_f[:])
            v_aug = persist.tile([128, NT, N_HEADS, DK + 1], F32R)
            res_all = [persist.tile([128, N], F32R, name=f"resall{p}")
                       for p in range(NPAIR)]

            for b in range(NB):
                if b > 0:
                    with nc.named_scope(f"b{b}_load"):
                        for ch in range(NCH):
                            nc.gpsimd.dma_start(
                                x_r[ch][:], x_d[b, ch * 128:(ch + 1) * 128, :])

                with nc.named_scope(f"b{b}_qkt"):
                    for p in range(NPAIR):
                        for qk in range(2):   # 0 = q, 1 = k
                            ps = ps_big.tile([128, N], F32, tag="big", name="qk_ps")
                            for nh in range(2):
                                nsl = slice(nh * 512, nh * 512 + 512)
                                for ch in range(NCH):
                                    nc.tensor.matmul(
                                        ps[:, nsl], wqk[ch][:, qk, p, :],
                                        x_r[ch][:, nsl],
                                        start=(ch == 0), stop=(ch == NCH - 1))
                            if qk == 0:
                                nc.vector.tensor_scalar(
                                    out=qpair[p][:], in0=ps[:],
                                    scalar1=bqkt[:, 0, p:p + 1],
                                    scalar2=None, op0=ALU.add)
                            else:
                                nc.vector.tensor_scalar(
                                    out=kpad[p][0][0:64, :], in0=ps[0:64, :],
                                    scalar1=bqkt[0:64, 1, p:p + 1],
                                    scalar2=None, op0=ALU.add)
                                nc.vector.tensor_scalar(
                                    out=kpad[p][1][64:128, :], in0=ps[64:128, :],
                                    scalar1=bqkt[64:128, 1, p:p + 1],
                                    scalar2=None, op0=ALU.add)

                with nc.named_scope(f"b{b}_v"):
                    for t in range(NT):
                        ps = ps_big.tile([128, N], F32, tag="big", name="v_ps")
                        for ch in range(NCH):
                            nc.tensor.matmul(
                                ps[:, 0:512], x_r[ch][:, t * 128:(t + 1) * 128],
                                wv[ch][:], start=(ch == 0), stop=(ch == NCH - 1))
                        nc.vector.tensor_add(
                            v_aug[:, t, :, 0:DK],
                            ps[:, 0:512].rearrange("p (h d) -> p h d", h=N_HEADS),
                            bv_bc[:].rearrange("p (h d) -> p h d", h=N_HEADS))
                        nc.vector.tensor_copy(
                            v_aug[:, t, :, DK:DK + 1], ones_f[:].unsqueeze(2))

                for p in range(NPAIR):
                    with nc.named_scope(f"b{b}_attn{p}"):
                        for ic in range(2):
                            isl = slice(ic * 512, ic * 512 + 512)
                            res_ps = [ps_res.tile([DK + 1, 512], F32, tag="res",
                                                  name=f"res_ps{s}")
                                      for s in range(2)]
                            for t in range(NT):
                                js = slice(t * 128, (t + 1) * 128)
                                s_ps = ps_big.tile([128, N], F32, tag="big",
                                                   name="s_ps")
                                nc.tensor.matmul(
                                    s_ps[:, 0:512], kpad[p][0][:, js],
                                    qpair[p][:, isl], start=True, stop=True)
                                nc.tensor.matmul(
                                    s_ps[:, 512:1024], kpad[p][1][:, js],
                                    qpair[p][:, isl], start=True, stop=True)
                                exp_sb = sbexp.tile([128, 2, 512], F32R, tag="exp",
                                                    name="exp_sb")
                                nc.scalar.activation(
                                    out=exp_sb[:], in_=s_ps[:], func=AF.Exp,
                                    scale=SCALE)
                                for s in range(2):
                                    nc.tensor.matmul(
                                        res_ps[s][:],
                                        v_aug[:, t, 2 * p + s, :],
                                        exp_sb[:, s, :],
                                        start=(t == 0), stop=(t == NT - 1))
                            # normalize: res / sumexp (broadcast row 64 via DRAM)
                            for s in range(2):
                                rcp_sb = sbwork.tile([1, 512], F32, tag="sums",
                                                     name="rcp_sb")
                                sum_sb = sbwork.tile([1, 512], F32, tag="sumsb",
                                                     name="sum_sb")
                                nc.vector.tensor_copy(sum_sb[:],
                                                      res_ps[s][DK:DK + 1, :])
                                nc.vector.reciprocal_approx_fast(
                                    out=rcp_sb[:], in_=sum_sb[:])
                                rcp_dram = dram.tile([1, 512], F32, tag="sumd",
                                                     name="rcp_dram")
                                nc.gpsimd.dma_start(rcp_dram[:], rcp_sb[:])
                                mult = sbwork.tile([64, 512], F32, tag="mult",
                                                   name="mult")
                                nc.gpsimd.dma_start(
                                    mult[:], rcp_dram[:].to_broadcast([64, 512]))
                                nc.vector.tensor_mul(
                                    res_all[p][s * 64:(s + 1) * 64, isl],
                                    res_ps[s][0:DK, :], mult[:])

                with nc.named_scope(f"b{b}_out"):
                    x_res_t = []
                    for ct in range(NCH):
                        xr = sbwork.tile([128, N], F32, tag="xres", bufs=4,
                                         name=f"x_res{ct}")
                        nc.sync.dma_start(xr[:], xpb_d[b, ct * 128:(ct + 1) * 128, :])
                        x_res_t.append(xr)
                    for ctp in range(NCH // 2):
                        cts = [2 * ctp, 2 * ctp + 1]
                        pss = [ps_big.tile([128, N], F32, tag="big",
                                           name=f"out_ps{ct}") for ct in cts]
                        for ch in range(NCH):
                            for i, ct in enumerate(cts):
                                csl = slice(ct * 128, (ct + 1) * 128)
                                for nh in range(2):
                                    nsl = slice(nh * 512, nh * 512 + 512)
                                    nc.tensor.matmul(
                                        pss[i][:, nsl], wo[ch][:, csl],
                                        res_all[ch][:, nsl],
                                        start=(ch == 0), stop=(ch == NCH - 1))
                        for i, ct in enumerate(cts):
                            csl = slice(ct * 128, (ct + 1) * 128)
                            out_sb = sbwork.tile([128, N], F32, tag="out",
                                                 name=f"out_sb{ct}")
                            nc.vector.tensor_add(out_sb[:], pss[i][:], x_res_t[ct][:])
                            nc.sync.dma_start(y_d[b, csl, :], out_sb[:])

    nc.finalize()
    return nc


_NC = None


def _get_nc():
    global _NC
    if _NC is None:
        _NC = build()
    return _NC


def make_in_maps(x, W_qkv, b_qkv, W_out, b_out):
    x = np.ascontiguousarray(np.asarray(x, np.float32)).reshape(16, C, N)
    b_out = np.asarray(b_out, np.float32)
    xpb = np.ascontiguousarray(x + b_out[None, :, None])
    w3 = np.asarray(W_qkv, np.float32).reshape(C, N_HEADS, 3, DK)
    w_qk = np.ascontiguousarray(
        np.stack([w3[:, :, 0], w3[:, :, 1]], axis=1).reshape(C, 2, NPAIR, 128))
    w_v = np.ascontiguousarray(w3[:, :, 2].reshape(C, C))
    b3 = np.asarray(b_qkv, np.float32).reshape(N_HEADS, 3, DK)
    # b_qk_t[j, qk, p] = bias for partition j of pair-tile (qk, p)
    b_qk_t = np.ascontiguousarray(
        np.stack([b3[:, 0], b3[:, 1]], axis=0)      # (2, 8, 64)
        .reshape(2, NPAIR, 128).transpose(2, 0, 1))  # (128, 2, 4)
    b_v = np.ascontiguousarray(b3[:, 2].reshape(1, C))
    maps = []
    for core in range(NCORES):
        maps.append({
            "x": x[core * NB:(core + 1) * NB],
            "xpb": xpb[core * NB:(core + 1) * NB],
            "w_qk": w_qk,
            "b_qk_t": b_qk_t,
            "w_v": w_v,
            "b_v": b_v,
            "w_out": np.asarray(W_out, np.float32),
        })
    return maps


def run_on_hw(in_maps, **kwargs):
    nc = _get_nc()
    return bass_utils.run_bass_kernel_spmd(
        nc, in_maps, core_ids=list(range(NCORES)), **kwargs)


def kernel(x, W_qkv, b_qkv, W_out, b_out):
    res = run_on_hw(make_in_maps(x, W_qkv, b_qkv, W_out, b_out))
    y = np.concatenate([r["y"] for r in res.results], axis=0)  # (16, C, N)
    return y.reshape(16, C, 32, 32).astype(np.float32)
